# revision 1
# baseline (speedup 1.0000x reference)
"""2-layer LSTM (B=64, S=512, E=H=1024) on 8 Trainium2 NeuronCores.

Strategy: tensor-parallel over the 4H gate dimension (each core owns a 128-wide
H-slice of each of the i,f,g,o gates = 512 gate columns per layer), both layers
pipelined one step apart so their batch-64 matmuls pack the 128-wide PE array
via column tiling. Per tick: PSUM accumulates gates for layer0(step t) on
partitions 0:64 and layer1(step t-1) on partitions 64:128; cell update runs
stacked; the new h slice pair is transposed (PE) and all-gathered to form the
next step's stationary h.T tiles. Layer-0 input projections are precomputed in
a sharded phase; layer-1 input projection is fused into the per-tick PSUM
accumulation (h0 @ W_ih1 slice); biases enter via identity-stationary inject
matmuls.
"""
import sys, types, os
import numpy as np

sys.path.insert(0, "/opt/trn_rl_repo")

import ml_dtypes

BF16 = ml_dtypes.bfloat16

B, S, E, H, L = 64, 512, 1024, 1024, 2
NC_ = 8          # cores
HS = 128         # H slice per core
GS = 4 * HS      # gate cols per core (512)
KT = H // 128    # 8 contraction tiles
P = 128

_cache = {}


def _install_axon_hooks():
    if "antenv.axon_hooks" in sys.modules:
        return
    sys.path.insert(0, "/root/.axon_site/trn_agent_boot")
    try:
        import trn_boot
        hook = trn_boot._ntff_profile_via_ctypes("/opt/axon/libaxon_pjrt.so")
    except Exception:
        hook = None
    mod = types.ModuleType("antenv.axon_hooks")
    mod._hook = hook
    mod.set_axon_ntff_profile_hook = lambda h: setattr(mod, "_hook", h)
    mod.get_axon_ntff_profile_hook = lambda: mod._hook
    sys.modules["antenv.axon_hooks"] = mod
    try:
        import antenv
        antenv.axon_hooks = mod
    except Exception:
        pass


def build(nsteps=S):
    import concourse.bass as bass
    import concourse.mybir as mybir
    import concourse.tile as tile
    from concourse import bacc

    dt = mybir.dt
    AF = mybir.ActivationFunctionType
    nc = bacc.Bacc(None, target_bir_lowering=False)

    BS = B * nsteps
    NBT = BS // P  # bs-tiles in proj phase

    # ---- inputs ----
    xt = nc.dram_tensor("xt", [E, BS], dt.bfloat16, kind="ExternalInput")
    wt0 = nc.dram_tensor("wt0", [H, GS], dt.bfloat16, kind="ExternalInput")
    whhT0 = nc.dram_tensor("whhT0", [H, GS], dt.bfloat16, kind="ExternalInput")
    whhT1 = nc.dram_tensor("whhT1", [H, GS], dt.bfloat16, kind="ExternalInput")
    wt1 = nc.dram_tensor("wt1", [H, GS], dt.bfloat16, kind="ExternalInput")
    b0bc = nc.dram_tensor("b0bc", [P, GS], dt.float32, kind="ExternalInput")
    b1bc = nc.dram_tensor("b1bc", [B, GS], dt.bfloat16, kind="ExternalInput")
    ht_init = nc.dram_tensor("ht_init", [P, KT * P], dt.bfloat16, kind="ExternalInput")
    hx1t_mine = nc.dram_tensor("hx1t_mine", [P, B], dt.bfloat16, kind="ExternalInput")
    cx_init = nc.dram_tensor("cx_init", [P, HS], dt.float32, kind="ExternalInput")
    ident = nc.dram_tensor("ident", [P, P], dt.bfloat16, kind="ExternalInput")

    # ---- outputs ----
    out_h1 = nc.dram_tensor("out_h1", [nsteps, B, HS], dt.float32, kind="ExternalOutput")
    out_fin = nc.dram_tensor("out_fin", [4, B, HS], dt.float32, kind="ExternalOutput")

    with tile.TileContext(nc) as tc:
        with (
            tc.tile_pool(name="wpool", bufs=1) as wpool,
            tc.tile_pool(name="state", bufs=1) as state,
            tc.tile_pool(name="xin", bufs=3) as xin,
            tc.tile_pool(name="gout", bufs=3) as gout,
            tc.tile_pool(name="g0in", bufs=4) as g0in,
            tc.tile_pool(name="epi", bufs=3) as epi,
            tc.tile_pool(name="psA", bufs=2, space="PSUM") as psA,
            tc.tile_pool(name="psT", bufs=2, space="PSUM") as psT,
            tc.tile_pool(name="dram", bufs=1, space="DRAM") as dram,
            tc.tile_pool(name="agd", bufs=4, space="DRAM") as agd,
        ):
            # persistent SBUF state
            w_wt0 = wpool.tile([P, KT, GS], dt.bfloat16, name="w_wt0")
            w_whh0 = wpool.tile([P, KT, GS], dt.bfloat16, name="w_whh0")
            w_whh1 = wpool.tile([P, KT, GS], dt.bfloat16, name="w_whh1")
            w_wt1 = wpool.tile([P, KT, GS], dt.bfloat16, name="w_wt1")
            sb_b0 = wpool.tile([P, GS], dt.float32, name="sb_b0")
            sb_b1 = wpool.tile([B, GS], dt.bfloat16, name="sb_b1")
            sb_id = wpool.tile([P, P], dt.bfloat16, name="sb_id")
            sb_hx1t = wpool.tile([P, B], dt.bfloat16, name="sb_hx1t")
            zb = wpool.tile([P, 1], dt.float32, name="zb")
            ht = [state.tile([P, KT, P], dt.bfloat16, name=f"ht{p}") for p in range(2)]
            cst = [state.tile([P, HS], dt.float32, name=f"cst{p}") for p in range(2)]

            r2 = lambda ap: ap.rearrange("(ko p) n -> p ko n", p=P)
            nc.sync.dma_start(w_wt0[:], r2(wt0[:]))
            nc.sync.dma_start(w_whh0[:], r2(whhT0[:]))
            nc.sync.dma_start(w_whh1[:], r2(whhT1[:]))
            nc.sync.dma_start(w_wt1[:], r2(wt1[:]))
            nc.sync.dma_start(sb_b0[:], b0bc[:])
            nc.sync.dma_start(sb_b1[:], b1bc[:])
            nc.sync.dma_start(sb_id[:], ident[:])
            nc.sync.dma_start(sb_hx1t[:], hx1t_mine[:])
            nc.sync.dma_start(ht[0][:], ht_init[:].rearrange("p (d c) -> p d c", c=P))
            nc.sync.dma_start(cst[0][:], cx_init[:])
            nc.gpsimd.memset(zb[:], 0.0)

            g0 = dram.tile([nsteps, B, GS], dt.bfloat16, name="g0")

            # ---------- phase A: layer-0 input projection ----------
            # two bs-tiles (256 rows) per DMA for efficient descriptors
            for i2 in range(NBT // 2):
                xt_sb = xin.tile([P, KT, 2 * P], dt.bfloat16, name="xt_sb")
                nc.sync.dma_start(
                    xt_sb[:], r2(xt[:, i2 * 2 * P:(i2 + 1) * 2 * P])
                )
                for j in range(2):
                    ps = psA.tile([P, GS], dt.float32, name="ps_proj")
                    for k in range(KT):
                        nc.tensor.matmul(
                            ps[:],
                            xt_sb[:, k, j * P:(j + 1) * P],
                            w_wt0[:, k, :],
                            start=(k == 0),
                            stop=(k == KT - 1),
                        )
                    gsb = gout.tile([P, GS], dt.bfloat16, name="gsb")
                    nc.vector.tensor_add(gsb[:], ps[:], sb_b0[:])
                    i = i2 * 2 + j
                    nc.sync.dma_start(
                        g0[2 * i:2 * i + 2, :, :].rearrange("t b n -> (t b) n"),
                        gsb[:],
                    )

            # ---------- phase B: fused two-layer recurrence ----------
            for t in range(nsteps + 1):
                par, nxt = t % 2, (t + 1) % 2
                cur = ht[par]
                has0 = t < nsteps     # layer-0 active this tick
                has1 = t >= 1         # layer-1 active (processes step t-1)
                ps = psA.tile([P, GS], dt.float32, name="ps_rec")
                if has0:
                    g0t = g0in.tile([B, GS], dt.bfloat16, name="g0t")
                    nc.sync.dma_start(g0t[:], g0[t, :, :])
                    nc.tensor.matmul(
                        ps[0:B, :], sb_id[0:B, 0:B], g0t[:],
                        start=True, stop=False, tile_position=(0, 0),
                    )
                if has1:
                    nc.tensor.matmul(
                        ps[B:P, :], sb_id[0:B, 0:B], sb_b1[:],
                        start=True, stop=False, tile_position=(0, B),
                    )
                for k in range(KT):
                    if has0:
                        nc.tensor.matmul(
                            ps[0:B, :], cur[:, k, 0:B], w_whh0[:, k, :],
                            start=False, stop=(k == KT - 1), tile_position=(0, 0),
                        )
                    if has1:
                        nc.tensor.matmul(
                            ps[B:P, :], cur[:, k, B:P], w_whh1[:, k, :],
                            start=False, stop=False, tile_position=(0, B),
                        )
                if has1:
                    for k in range(KT):
                        nc.tensor.matmul(
                            ps[B:P, :], cur[:, k, 0:B], w_wt1[:, k, :],
                            start=False, stop=(k == KT - 1), tile_position=(0, B),
                        )

                lo, hi = (0 if has0 else B), (P if has1 else B)
                sl = slice(lo, hi)
                sig_i = epi.tile([P, HS], dt.float32, name="sig_i")
                sig_f = epi.tile([P, HS], dt.float32, name="sig_f")
                tanh_g = epi.tile([P, HS], dt.float32, name="tanh_g")
                sig_o = epi.tile([P, HS], dt.float32, name="sig_o")
                nc.scalar.activation(tanh_g[sl, :], ps[sl, 2 * HS:3 * HS], AF.Tanh, bias=zb[sl, :])
                nc.scalar.activation(sig_f[sl, :], ps[sl, HS:2 * HS], AF.Sigmoid, bias=zb[sl, :])
                nc.scalar.activation(sig_i[sl, :], ps[sl, 0:HS], AF.Sigmoid, bias=zb[sl, :])
                nc.scalar.activation(sig_o[sl, :], ps[sl, 3 * HS:4 * HS], AF.Sigmoid, bias=zb[sl, :])
                t1 = epi.tile([P, HS], dt.float32, name="t1")
                t2 = epi.tile([P, HS], dt.float32, name="t2")
                nc.vector.tensor_mul(t2[sl, :], sig_f[sl, :], cst[par][sl, :])
                nc.vector.tensor_mul(t1[sl, :], sig_i[sl, :], tanh_g[sl, :])
                nc.vector.tensor_add(cst[nxt][sl, :], t1[sl, :], t2[sl, :])
                if t == nsteps - 1 or t == nsteps:
                    # keep untouched half of c for finals bookkeeping
                    if lo == B:
                        pass
                tanh_c = epi.tile([P, HS], dt.float32, name="tanh_c")
                nc.scalar.activation(tanh_c[sl, :], cst[nxt][sl, :], AF.Tanh, bias=zb[sl, :])
                h_f32 = epi.tile([P, HS], dt.float32, name="h_f32")
                nc.vector.tensor_mul(h_f32[sl, :], sig_o[sl, :], tanh_c[sl, :])

                if has1:
                    nc.sync.dma_start(out_h1[t - 1, :, :], h_f32[B:P, :])
                if t == nsteps - 1:
                    nc.sync.dma_start(out_fin[0, :, :], h_f32[0:B, :])
                    nc.sync.dma_start(out_fin[2, :, :], cst[nxt][0:B, :])
                if t == nsteps:
                    nc.sync.dma_start(out_fin[1, :, :], h_f32[B:P, :])
                    nc.sync.dma_start(out_fin[3, :, :], cst[nxt][B:P, :])
                    break  # no exchange needed

                # carry forward the half of c not updated this tick
                if not has1:
                    nc.vector.tensor_copy(cst[nxt][B:P, :], cst[par][B:P, :])

                # ---- transpose h slice pair and exchange ----
                h_bf = epi.tile([P, HS], dt.bfloat16, name="h_bf")
                nc.vector.tensor_copy(h_bf[sl, :], h_f32[sl, :])
                if not has1:
                    # seed layer-1 columns with initial hx[1].T on tick 0
                    pass
                pst = psT.tile([P, P], dt.bfloat16, name="pst")
                nc.tensor.transpose(pst[:], h_bf[:], sb_id[:])
                exch = epi.tile([P, P], dt.bfloat16, name="exch")
                if has1:
                    nc.vector.tensor_copy(exch[:], pst[:])
                else:
                    nc.vector.tensor_copy(exch[:, 0:B], pst[:, 0:B])
                    nc.vector.tensor_copy(exch[:, B:P], sb_hx1t[:])

                ag_in = agd.tile([P, P], dt.bfloat16, name="ag_in")
                ag_out = agd.tile([NC_ * P, P], dt.bfloat16, name="ag_out")
                nc.sync.dma_start(ag_in[:], exch[:])
                nc.gpsimd.collective_compute(
                    "AllGather",
                    mybir.AluOpType.bypass,
                    replica_groups=[list(range(NC_))],
                    ins=[ag_in.opt()],
                    outs=[ag_out.opt()],
                )
                nc.sync.dma_start(
                    ht[nxt][:], ag_out[:].rearrange("(d p) c -> p d c", p=P)
                )
    nc.compile()
    return nc


def _prep(inputs, nsteps=S):
    input_ = np.asarray(inputs["input_"], dtype=np.float32)[:, :nsteps]
    hx = np.asarray(inputs["hx"], dtype=np.float32)
    cx = np.asarray(inputs["cx"], dtype=np.float32)
    W_ih = np.asarray(inputs["W_ih"], dtype=np.float32)
    W_hh = np.asarray(inputs["W_hh"], dtype=np.float32)
    b_ih = np.asarray(inputs["b_ih"], dtype=np.float32)
    b_hh = np.asarray(inputs["b_hh"], dtype=np.float32)

    # [E, S*B] with bs index = t*B + b
    xt = np.ascontiguousarray(input_.transpose(2, 1, 0).reshape(E, nsteps * B)).astype(BF16)
    bias = b_ih + b_hh  # [L, 4H]

    in_maps = []
    for c in range(NC_):
        cols = np.concatenate([np.arange(g * H + c * HS, g * H + (c + 1) * HS) for g in range(4)])
        wt0 = np.ascontiguousarray(W_ih[0][cols, :].T).astype(BF16)          # [E, GS]
        whhT0 = np.ascontiguousarray(W_hh[0][cols, :].T).astype(BF16)
        whhT1 = np.ascontiguousarray(W_hh[1][cols, :].T).astype(BF16)
        wt1 = np.ascontiguousarray(W_ih[1][cols, :].T).astype(BF16)
        b0 = bias[0][cols]
        b1 = bias[1][cols]
        b0bc = np.broadcast_to(b0, (P, GS)).astype(np.float32).copy()
        b1bc = np.broadcast_to(b1, (B, GS)).astype(BF16).copy()
        # initial stationary: slot d = [hx0.T rows 128d.. | hx1.T rows 128d..]
        h0T = hx[0].T.astype(BF16)  # [H, B]
        h1T = hx[1].T.astype(BF16)
        ht_init = np.zeros((P, KT * P), dtype=BF16)
        for d in range(KT):
            ht_init[:, d * P:d * P + B] = h0T[d * P:(d + 1) * P, :]
            ht_init[:, d * P + B:(d + 1) * P] = h1T[d * P:(d + 1) * P, :]
        hx1t_mine = np.ascontiguousarray(h1T[c * HS:(c + 1) * HS, :])  # [128, 64]
        cx_init = np.concatenate(
            [cx[0][:, c * HS:(c + 1) * HS], cx[1][:, c * HS:(c + 1) * HS]], axis=0
        ).astype(np.float32)  # [128, 128]
        ident = np.eye(P, dtype=BF16)
        in_maps.append({
            "xt": xt, "wt0": wt0, "whhT0": whhT0, "whhT1": whhT1, "wt1": wt1,
            "b0bc": b0bc, "b1bc": b1bc, "ht_init": ht_init,
            "hx1t_mine": hx1t_mine, "cx_init": cx_init, "ident": ident,
        })
    return in_maps


def _assemble(results, nsteps=S):
    out = np.zeros((B, nsteps, H), dtype=np.float32)
    hxf = np.zeros((L, B, H), dtype=np.float32)
    cxf = np.zeros((L, B, H), dtype=np.float32)
    for c in range(NC_):
        h1 = results[c]["out_h1"]          # [S, B, HS]
        fin = results[c]["out_fin"]        # [4, B, HS]
        out[:, :, c * HS:(c + 1) * HS] = h1.transpose(1, 0, 2)
        hxf[0, :, c * HS:(c + 1) * HS] = fin[0]
        hxf[1, :, c * HS:(c + 1) * HS] = fin[1]
        cxf[0, :, c * HS:(c + 1) * HS] = fin[2]
        cxf[1, :, c * HS:(c + 1) * HS] = fin[3]
    return out, hxf, cxf


def run(inputs, nsteps=S, trace=False):
    _install_axon_hooks()
    from concourse import bass_utils
    key = nsteps
    if key not in _cache:
        _cache[key] = build(nsteps)
    nc = _cache[key]
    in_maps = _prep(inputs, nsteps)
    res = bass_utils.run_bass_kernel_spmd(
        nc, in_maps, core_ids=list(range(NC_)), trace=trace
    )
    return _assemble(res.results, nsteps) + ((res.exec_time_ns,) if trace else ())


def kernel(**inputs):
    out, hxf, cxf = run(inputs, S, trace=False)[:3]
    return out, hxf, cxf
